# revision 27
# baseline (speedup 1.0000x reference)
"""Contrastive-loss kernel for Trainium2, 8 NeuronCores — r2-scaled feature-Gram.

Math
----
reference:
    yn  = ys / clip(||ys||, 1e-6)          (row-normalize)
    cos = yn @ yn.T                         [B, B]
    pair_loss = same ? relu(2 - cos)^2 : cos^2
    loss = sum(strict_lower(pair_loss)) / (B*(B-1)/2)

Since margin M = 2 and cos <= 1: relu(2-cos)^2 = (2-cos)^2 = cos^2 + 4*(1-cos),
so summing over the strict lower triangle (diagonal terms: cos_ii = 1):
    sum_{i>j} pair_loss = (F1 - B)/2 + 2*(N2 - SS)
with
    F1 = sum_ij cos_ij^2 = ||Yn^T Yn||_F^2    (feature Gram, D x D)
    N2 = sum_c n_c^2 = ||O^T O||_F^2          (O = one-hot labels [B, 10])
    SS = sum_c ||S_c||^2 = ||O^T Yn||_F^2     (S_c = sum of yn rows, label c)

Key identities that kill the collectives (the entire baseline bottleneck —
its three DRAM collectives cost 272us in the cost model):

  * M = Yn^T Yn = X^T R^2 X  with R = diag(1/||x_r||): only the LHS of the
    matmul needs the row scaling — the rhs is the RAW fp8 matrix straight
    from the host.  Per-core lhsT = own 256 feature columns (host rotates
    columns so each core's slice sits at 0:256) scaled by r32^2.
  * S-slice: S[:, own 256 features] = (R O)^T X[:, own] — lhsT = r-scaled
    one-hot, rhs = raw own-feature columns.  sum over cores of
    ||S_slice||^2 = SS exactly (features partition the columns).
  * N2 = ||O^T O||_F^2 computed redundantly on every core from the raw
    one-hot (exact integers).

The ONLY cross-core data needed is r32 = 32/||x_r|| for all 4096 rows:
one AllGather of [128, 4] f32 per core -> [1024, 4] (16 KB), ~15.4us.
Everything else (8.4 MB raw fp8 rows) comes from the host via DMA, which
runs on otherwise-idle queues concurrently with the norm/gather phase.

Device schedule (SPMD, 8 cores):
  SP   : ys tiles 0,2 -> xplus pair-chunks (16 x 1.6us) -> out DMAs
  Pool : ys tiles 1,3, one-hot, memsets, collective
  ACT  : act-table preload, Square(t0), Square(t3), sqrt, cc_in DMA,
         lhsT odd planes, aux/H1/H2 epilogues
  DVE  : TTR(t1), TTR(t2), eps-clip/recip/x32, gather readback, r32^2,
         lhsT even planes + RO planes, final adds
  PE   : optional warm-up matmuls (keeps the p-state ramp hot during the
         collective), then H1 = dsub0 Gram + S + OTO interleaved (PSUM
         banks 0-3 + one aux bank), then H2 = dsub1 (banks 4-7 after the
         tiny aux epilogue frees them).
"""

import os
import sys

for _p in ("/opt/trn_rl_repo", "/root/.axon_site/_ro/trn_rl_repo"):
    if _p not in sys.path and os.path.isdir(_p):
        sys.path.append(_p)

import numpy as np

import concourse.bass as bass
import concourse.mybir as mybir
import concourse.tile as tile
from concourse.bass import ds, ts  # noqa: F401

F32 = mybir.dt.float32
BF16 = mybir.dt.bfloat16
FP8 = mybir.dt.float8e4
AF = mybir.ActivationFunctionType
ALU = mybir.AluOpType
DR = mybir.MatmulPerfMode.DoubleRow

P = 128       # partitions
NCLS = 10     # label classes
NCW = 16      # one-hot tile width (padded: 10-wide fp8 DR ldweights fails walrus)
SCALE = 32.0  # fp8 pre-scale on r32 = SCALE/||row||
NJUNK = 111   # PE warm-up matmuls during the collective (tuned empirically)


def _split_multi_waits(nc):
    """Split instructions carrying >1 semaphore wait.

    The walrus in this environment rejects compute instructions with more
    than one sync-wait command ("Too many sync wait commands"). Move the
    extra waits onto standalone EventSemaphore instructions inserted just
    before, on the same engine — semantically identical (the engine's
    sequencer blocks on each in order).
    """
    n_split = 0
    for fn in nc.m.functions:
        for bb in fn.blocks:
            new_insts = []
            for ins in bb.instructions:
                si = ins.sync_info
                if (
                    si is not None
                    and len(si.on_wait) > 1
                    and not isinstance(ins, mybir.InstEventSemaphore)
                ):
                    extra = list(si.on_wait[1:])
                    ins.sync_info = mybir.SyncInfo(
                        on_wait=[si.on_wait[0]], on_update=list(si.on_update)
                    )
                    for w in extra:
                        n_split += 1
                        ev = mybir.InstEventSemaphore(
                            name=f"antsplitwait_{n_split}_{ins.name}",
                            engine=ins.engine,
                            ins=[],
                            outs=[],
                            sync_info=mybir.SyncInfo(on_wait=[w], on_update=[]),
                            bass_nofuse=True,
                        )
                        new_insts.append(ev)
                new_insts.append(ins)
            bb.instructions = new_insts
    return n_split


def build_gram_loss(B=4096, D=2048, C=8, S=4):
    """Build the SPMD bass program (one nc, run on C cores). S unused."""
    assert B == 4096 and D == 2048 and C == 8
    Bs = B // C          # 512 rows per core
    RT = Bs // P         # 4 own row-tiles
    KC = B // P          # 32 row planes of 128
    NPAIR = KC // 2      # 16 DoubleRow plane pairs
    DC = D // C          # 256 own feature columns
    JW = 512             # matmul j-block width (one PSUM bank)

    nc = bass.Bass(num_devices=C)

    ys_mine = nc.dram_tensor("ys_mine", [Bs, D], BF16, kind="ExternalInput")
    xplus = nc.dram_tensor("xplus", [P, KC * D], FP8, kind="ExternalInput")
    oh_all = nc.dram_tensor("oh_all", [P, KC * NCW], FP8, kind="ExternalInput")
    out_parts = nc.dram_tensor("out_parts", [P, 10], F32, kind="ExternalOutput")

    cc_in = nc.dram_tensor("cc_in", [P, RT], F32)
    cc_out = nc.dram_tensor("cc_out", [C * P, RT], F32, addr_space="Shared")

    with tile.TileContext(nc) as tc:
        with (
            tc.tile_pool(name="const", bufs=1) as const_pool,
            tc.tile_pool(name="big", bufs=1) as big_pool,
            tc.tile_pool(name="ysin", bufs=1) as ys_pool,
            tc.tile_pool(name="sqscr", bufs=1) as sq_pool,
            tc.tile_pool(name="small", bufs=4) as small_pool,
            tc.tile_pool(name="ep", bufs=1) as ep_pool,
            tc.tile_pool(name="mm", bufs=1, space="PSUM") as mm_psum,
        ):
            # ---------------- tiles ----------------
            xp_sb = big_pool.tile([P, KC, D], FP8)
            lhsT_sb = big_pool.tile([P, KC, DC], FP8)
            oh_sb = const_pool.tile([P, KC, NCW], FP8)
            ro_sb = const_pool.tile([P, KC, NCW], FP8)
            r32g = const_pool.tile([P, KC], F32)
            r32sq = const_pool.tile([P, KC], F32)

            ys_t = [ys_pool.tile([P, D], BF16, name=f"ys_t{t}") for t in range(RT)]
            sq_a = sq_pool.tile([P, D], BF16)   # ACT square scratch
            sq_d = sq_pool.tile([P, D], BF16)   # DVE square scratch
            sq_d2 = sq_pool.tile([P, D], BF16)  # DVE accum-sum scratch
            ssq = small_pool.tile([P, RT], F32)
            norm_t = small_pool.tile([P, RT], F32)
            rcp_t = small_pool.tile([P, RT], F32)
            r32_own = small_pool.tile([P, RT], F32)
            dummy = small_pool.tile([P, 1], F32)
            dummy_o = small_pool.tile([P, 1], BF16)
            out_sb = const_pool.tile([P, 2], F32)

            # ---------------- t=0: DMAs + ACT table preload ----------------
            # SP: ys tiles 0,2 then the 16 xplus pair-chunks.
            nc.sync.dma_start(out=ys_t[0][:], in_=ys_mine[ts(0, P), :])
            nc.sync.dma_start(out=ys_t[2][:], in_=ys_mine[ts(2, P), :])
            for g in range(NPAIR):
                nc.sync.dma_start(
                    out=xp_sb[:, 2 * g : 2 * g + 2, :],
                    in_=xplus[:, 2 * g * D : (2 * g + 2) * D].rearrange(
                        "p (k d) -> p k d", k=2
                    ),
                )
            # Pool: ys tiles 1,3 + one-hot + memsets.
            nc.gpsimd.memset(dummy[:], 1.0)
            nc.gpsimd.dma_start(out=ys_t[1][:], in_=ys_mine[ts(1, P), :])
            nc.gpsimd.dma_start(out=ys_t[3][:], in_=ys_mine[ts(3, P), :])
            nc.gpsimd.dma_start(out=oh_sb[:, :, :], in_=oh_all[:, :].rearrange(
                "p (k c) -> p k c", c=NCW
            ))
            nc.vector.memset(out_sb[:], 0.0)

            # ACT: pay the Square activation-table load before data arrives.
            nc.scalar.activation(dummy_o[:], dummy[:], AF.Square)

            # ---------------- norms of own 512 rows ----------------
            # ACT squares tiles 0,3; DVE squares tiles 1,2 (TTR mult+add).
            for t, eng in ((0, "act"), (1, "dve"), (3, "act"), (2, "dve")):
                if eng == "act":
                    nc.scalar.activation(
                        sq_a[:], ys_t[t][:], AF.Square,
                        accum_out=ssq[:, t : t + 1],
                    )
                else:
                    nc.vector.tensor_tensor(
                        sq_d[:], ys_t[t][:], ys_t[t][:], ALU.mult
                    )
                    nc.vector.tensor_scalar(
                        sq_d2[:], sq_d[:], 1.0, None, ALU.mult, ALU.add,
                        accum_out=ssq[:, t : t + 1],
                    )
            # O^T O chain needs only host data — run it pre-gather so its
            # epilogue is long done before the main phases want the banks.
            ps_oto = mm_psum.tile([P, JW], F32, tag="b5", name="ps_oto")
            for g in range(NPAIR):
                pr = slice(2 * g, 2 * g + 2)
                nc.tensor.matmul(
                    ps_oto[0:NCW, 0:NCW],
                    lhsT=oh_sb[:, pr, :],
                    rhs=oh_sb[:, pr, :],
                    start=g == 0,
                    stop=g == NPAIR - 1,
                    perf_mode=DR,
                )

            # PE p-state warm-up: the ramp resets on idle, so keep the PE
            # streaming junk during the collective; the real Gram stream then
            # starts at full clock. Tuned so the junk drains right as the
            # lhsT builds land.
            if NJUNK:
                ps_junk = mm_psum.tile([P, JW], F32, tag="b6", name="ps_junk")
                for _ in range(NJUNK):
                    nc.tensor.matmul(
                        ps_junk[0:4, :],
                        lhsT=ys_t[0][:, 0:4],
                        rhs=ys_t[0][:, 0:JW],
                        start=True,
                        stop=True,
                    )
            # OTO epilogue on DVE (idle during the collective; ACT's in-order
            # queue must stay clear for sqrt -> cc_in -> readback).
            o_cpy = ep_pool.tile([NCW, NCW], F32)
            o_sq = ep_pool.tile([NCW, NCW], F32)
            o_s2 = ep_pool.tile([NCW, NCW], F32)
            n2_acc = small_pool.tile([NCW, 1], F32)
            nc.vector.tensor_copy(o_cpy[:], ps_oto[0:NCW, 0:NCW])
            nc.vector.tensor_tensor(o_sq[:], o_cpy[:], o_cpy[:], ALU.mult)
            nc.vector.tensor_scalar(
                o_s2[:], o_sq[:], 1.0, None, ALU.mult, ALU.add,
                accum_out=n2_acc[:],
            )

            nc.scalar.sqrt(norm_t[:], ssq[:])
            nc.vector.tensor_scalar_max(norm_t[:], norm_t[:], 1e-6)
            nc.vector.reciprocal(rcp_t[:], norm_t[:])
            nc.vector.tensor_scalar_mul(r32_own[:], rcp_t[:], SCALE)

            # ---------------- the one collective: AllGather r32 ----------
            nc.scalar.dma_start(out=cc_in[:, :], in_=r32_own[:])
            nc.gpsimd.collective_compute(
                "AllGather",
                ALU.bypass,
                replica_groups=[list(range(C))],
                ins=[cc_in[:, :]],
                outs=[cc_out[:, :]],
            )
            # readback on ACT (idle during the collective; DVE cannot issue
            # DMAs). cc_out[(r p), t] -> [p, r, t]: column index r*4 + t =
            # global plane kc.
            nc.scalar.dma_start(
                out=r32g[:, :].rearrange("p (r t) -> p r t", t=RT),
                in_=cc_out[:, :].rearrange("(r p) t -> p r t", p=P),
            )
            nc.vector.tensor_tensor(r32sq[:], r32g[:], r32g[:], ALU.mult)

            # ---------------- lhsT + RO builds (pipelined under PE) -------
            # DVE: even planes + both RO planes per pair; ACT: odd planes.
            for g in range(NPAIR):
                k0, k1 = 2 * g, 2 * g + 1
                nc.vector.tensor_scalar_mul(
                    lhsT_sb[:, k0, :], xp_sb[:, k0, 0:DC], r32sq[:, k0 : k0 + 1]
                )
                nc.vector.tensor_scalar_mul(
                    ro_sb[:, k0, :], oh_sb[:, k0, :], r32g[:, k0 : k0 + 1]
                )
                nc.vector.tensor_scalar_mul(
                    ro_sb[:, k1, :], oh_sb[:, k1, :], r32g[:, k1 : k1 + 1]
                )
                nc.scalar.mul(
                    lhsT_sb[:, k1, :], xp_sb[:, k1, 0:DC], r32sq[:, k1 : k1 + 1]
                )

            # ---------------- PE: three phases over per-bank tiles --------
            # P1 = dsub0 x j0-3 (banks 0-3) + S slice (bank 7)
            # P2 = dsub1 x j0-2 (banks 4-6)  — epilogue of P1 runs under it
            # P3 = dsub1 x j3   (bank 7, after the tiny S epilogue)
            pb = [
                mm_psum.tile([P, JW], F32, tag=f"b{j}", name=f"pb{j}")
                for j in range(7)
            ]
            ps_s = mm_psum.tile([P, JW], F32, tag="b7", name="ps_s")

            for g in range(NPAIR):
                pr = slice(2 * g, 2 * g + 2)
                st, sp = g == 0, g == NPAIR - 1
                for j in range(4):
                    nc.tensor.matmul(
                        pb[j][:, :],
                        lhsT=lhsT_sb[:, pr, 0:P],
                        rhs=xp_sb[:, pr, ts(j, JW)],
                        start=st,
                        stop=sp,
                        perf_mode=DR,
                    )
                nc.tensor.matmul(
                    ps_s[0:NCW, 0:DC],
                    lhsT=ro_sb[:, pr, :],
                    rhs=xp_sb[:, pr, 0:DC],
                    start=st,
                    stop=sp,
                    perf_mode=DR,
                )

            # P1 epilogues: SS partial first on ACT (it gates P3), then F1
            # bank squares split ACT/DVE, all running under P2.
            s_scr = ep_pool.tile([NCW, DC], BF16)
            s_acc = small_pool.tile([NCW, 1], F32)
            nc.scalar.activation(
                s_scr[:], ps_s[0:NCW, 0:DC], AF.Square, accum_out=s_acc[:]
            )
            nc.scalar.copy(out_sb[0:NCLS, 0:1], s_acc[0:NCLS, :])
            nc.scalar.copy(out_sb[0:NCLS, 1:2], n2_acc[0:NCLS, :])

            red_all = const_pool.tile([P, 8], F32)
            a_scr = ep_pool.tile([P, JW], BF16)
            d_cpy = ep_pool.tile([P, JW], BF16)
            d_sq = ep_pool.tile([P, JW], BF16)
            d_s2 = ep_pool.tile([P, JW], BF16)

            def ep_act(j):
                nc.scalar.activation(
                    a_scr[:], pb[j][:, :], AF.Square,
                    accum_out=red_all[:, j : j + 1],
                )

            def ep_dve(j, src=None):
                src = pb[j][:, :] if src is None else src
                nc.vector.tensor_copy(d_cpy[:], src)
                nc.vector.tensor_tensor(d_sq[:], d_cpy[:], d_cpy[:], ALU.mult)
                nc.vector.tensor_scalar(
                    d_s2[:], d_sq[:], 1.0, None, ALU.mult, ALU.add,
                    accum_out=red_all[:, j : j + 1],
                )

            ep_act(0)
            ep_act(1)
            ep_dve(2)
            ep_dve(3)

            # P2
            for g in range(NPAIR):
                pr = slice(2 * g, 2 * g + 2)
                st, sp = g == 0, g == NPAIR - 1
                for j in range(3):
                    nc.tensor.matmul(
                        pb[4 + j][:, :],
                        lhsT=lhsT_sb[:, pr, P:DC],
                        rhs=xp_sb[:, pr, ts(j, JW)],
                        start=st,
                        stop=sp,
                        perf_mode=DR,
                    )

            # mid-run output DMA (SS partial + N2), and P2 epilogues under P3
            nc.sync.dma_start(out=out_parts[:, 8:10], in_=out_sb[:, 0:2])
            ep_act(4)
            ep_act(5)
            ep_dve(6)

            # P3
            ps_p3 = mm_psum.tile([P, JW], F32, tag="b7", name="ps_p3")
            for g in range(NPAIR):
                pr = slice(2 * g, 2 * g + 2)
                nc.tensor.matmul(
                    ps_p3[:, :],
                    lhsT=lhsT_sb[:, pr, P:DC],
                    rhs=xp_sb[:, pr, ts(3, JW)],
                    start=g == 0,
                    stop=g == NPAIR - 1,
                    perf_mode=DR,
                )

            # tail: one bank square, then the red columns out
            nc.scalar.activation(
                a_scr[:], ps_p3[:, :], AF.Square,
                accum_out=red_all[:, 7:8],
            )
            nc.sync.dma_start(out=out_parts[:, 0:8], in_=red_all[:, :])

    _split_multi_waits(nc)
    return nc


def make_in_maps(ys, labels, B, D, C, S=4):
    """Shard host inputs into per-core input maps (dtype packing only)."""
    import ml_dtypes

    ys = np.ascontiguousarray(ys, dtype=np.float32)
    labels = np.asarray(labels).astype(np.int64)
    Bs = B // C
    KC = B // P
    DC = D // C
    ys_f8 = ys.astype(ml_dtypes.float8_e4m3)
    ys_bf = ys.astype(ml_dtypes.bfloat16)

    # one-hot in plane layout [p, kc*10 + c] — identical on every core
    lab2 = labels.reshape(KC, P)
    oh = np.zeros((P, KC, NCW), dtype=ml_dtypes.float8_e4m3)
    oh[np.arange(P)[:, None], np.arange(KC)[None, :], lab2.T] = 1.0
    oh = np.ascontiguousarray(oh.reshape(P, KC * NCW))

    in_maps = []
    for k in range(C):
        # rotate columns so core k's 256 features sit at 0:DC, then pack
        # rows into [p, kc*D + d] with global row = kc*128 + p
        rot = np.concatenate(
            [ys_f8[:, k * DC :], ys_f8[:, : k * DC]], axis=1
        )
        xp = np.ascontiguousarray(
            rot.reshape(KC, P, D).transpose(1, 0, 2).reshape(P, KC * D)
        )
        in_maps.append(
            {
                "ys_mine": np.ascontiguousarray(ys_bf[k * Bs : (k + 1) * Bs]),
                "xplus": xp,
                "oh_all": oh,
            }
        )
    return in_maps


def combine_parts(parts_list, B):
    """parts_list: per-core [128, 10] f32 partials -> scalar loss.

    cols 0-7 = F1 bank partials; col 8 = SS slice partial; col 9 = N2.
    """
    f1 = 0.0
    ss = 0.0
    for p in parts_list:
        p = np.asarray(p, dtype=np.float64)
        f1 += p[:, 0:8].sum()
        ss += p[0:NCLS, 8].sum()
    f1 /= SCALE**4
    ss /= SCALE**2
    n2 = np.asarray(parts_list[0], dtype=np.float64)[0:NCLS, 9].sum()
    total = (f1 - B) / 2.0 + 2.0 * (n2 - ss)
    n_pair = B * (B - 1) // 2
    return np.float32(total / n_pair)


_CACHED = {}


def kernel(ys: np.ndarray, labels: np.ndarray) -> np.ndarray:
    B, D = ys.shape
    C = 8
    key = (B, D, C)
    if key not in _CACHED:
        _CACHED[key] = build_gram_loss(B=B, D=D, C=C)
    nc = _CACHED[key]

    from concourse.bass_utils import run_bass_kernel_spmd

    in_maps = make_in_maps(np.asarray(ys), np.asarray(labels), B, D, C)
    res = run_bass_kernel_spmd(nc, in_maps, core_ids=list(range(C)))
    parts = [res.results[i]["out_parts"] for i in range(C)]
    return combine_parts(parts, B)


if __name__ == "__main__":
    nc = build_gram_loss()
    print("built ok:", len(nc.m.functions[0].blocks), "blocks")
